# revision 1
# baseline (speedup 1.0000x reference)
"""Trainium2 Bass kernel for nn_ConvCapsuleLayer3D.

Self-contained: takes FULL inputs x[32,32,32,8,16], W[16,3,3,1,256], b[16,16,1,1],
returns FULL output [32,30,30,16,16] (fp32). Data-parallel over batch across 8
NeuronCores (4 samples each).

Per-sample plan (all fp32):
  conv:   im2col [144,(d,hw)=7200] built by 72 DMAs straight from HBM; matmul
          with im2col slices as stationary operand -> votes [hw_chunk, (i,o,a)]
          in PSUM, drained to SBUF by ScalarE.
  routing (3 iters), layout = hw on partitions (h-aligned chunks of 120/60):
          softmax over o, preact = sum_i r*V (DVE mul + DVE reduce), bias add,
          squash over w via tiny mask matmuls on TensorE (sum over the 30
          w-positions that live inside each chunk's partitions), act = preact *
          scale, agreement logits update (DVE mul + reduce over a).
"""
import os
import sys

import numpy as np

sys.path.insert(0, "/opt/trn_rl_repo")

# --- problem constants (hardcoded; kernel.py must not read /root/problem) ---
B, H, WD, IC, IA = 32, 32, 32, 8, 16
OC, NA = 16, 16
K = 3
HC, WC = H - K + 1, WD - K + 1       # 30, 30
HW = HC * WC                         # 900
CO = OC * NA                         # 256
NCORES = 8
NSAMP = B // NCORES                  # 4
EPS = 1e-7
ROUTINGS = 3

CP_FULL = 120                        # 4 h-rows per chunk
CHUNKS = [(c, CP_FULL, 4) for c in range(7)] + [(7, 60, 2)]  # (c, cp, nj)


def _build_body(ctx, tc, x_ap, w_ap, b_ap, out_ap):
    import concourse.bass as bass
    import concourse.mybir as mybir

    nc = tc.nc
    f32 = mybir.dt.float32
    Alu = mybir.AluOpType
    Act = mybir.ActivationFunctionType
    X = mybir.AxisListType.X

    def pap(t, part, dims, off=0):
        """AP over tile t: partitions [0,part), free dims [[step,count],...] (elements)."""
        a = t if isinstance(t, bass.AP) else t.ap()
        pstep = a.ap[0][0]  # partition pitch in elements (may be padded)
        return bass.AP(tensor=a.tensor, offset=a.offset + off,
                       ap=[[pstep, part]] + dims)

    consts = ctx.enter_context(tc.tile_pool(name="consts", bufs=1))
    imc_pool = ctx.enter_context(tc.tile_pool(name="imc", bufs=2))
    v_pool = ctx.enter_context(tc.tile_pool(name="votes", bufs=3))
    vr_pool = ctx.enter_context(tc.tile_pool(name="vr", bufs=3))
    small = ctx.enter_context(tc.tile_pool(name="small", bufs=4))
    acts = ctx.enter_context(tc.tile_pool(name="acts", bufs=4))
    psum_c = ctx.enter_context(tc.tile_pool(name="psc", bufs=4, space="PSUM"))
    psum_s = ctx.enter_context(tc.tile_pool(name="pss", bufs=2, space="PSUM"))
    psum_b = ctx.enter_context(tc.tile_pool(name="psb", bufs=2, space="PSUM"))

    # ---- constants ----
    wa = consts.tile([128, CO], f32, tag="wa")      # K rows (kh,kw,kd) 0..127
    wb = consts.tile([16, CO], f32, tag="wb")       # K rows 128..143
    for kh in range(K):
        for kw in range(K):
            blk = kh * K + kw
            src = bass.AP(tensor=w_ap.tensor, offset=w_ap.offset + kh * 768 + kw * 256,
                          ap=[[2304, 16], [1, 256]])
            if blk < 8:
                nc.sync.dma_start(out=wa[blk * 16:(blk + 1) * 16, :], in_=src)
            else:
                nc.sync.dma_start(out=wb[:, :], in_=src)

    bfull = consts.tile([128, CO], f32, tag="bfull")
    nc.sync.dma_start(out=bfull[:, :],
                      in_=bass.AP(tensor=b_ap.tensor, offset=b_ap.offset,
                                  ap=[[0, 128], [1, 256]]))

    zero_t = consts.tile([128, 1], f32, tag="zero")
    nc.vector.memset(zero_t[:, :], 0.0)
    eps_t = consts.tile([128, 1], f32, tag="eps")
    nc.vector.memset(eps_t[:, :], EPS)

    # mask[p,j] = (p//30 == j); sel[j,p] = (p//30 == j). Engine writes must
    # start at partition 0 (32-aligned), so build via iota(p-30j) + compares.
    i32 = mybir.dt.int32
    mask = consts.tile([CP_FULL, 4], f32, tag="mask")
    sel = consts.tile([4, CP_FULL], f32, tag="sel")
    mi = consts.tile([CP_FULL, 4], i32, tag="mi")
    si = consts.tile([4, CP_FULL], i32, tag="si")
    mf = consts.tile([CP_FULL, 4], f32, tag="mf")
    sf = consts.tile([4, CP_FULL], f32, tag="sf")
    nc.gpsimd.iota(mi[:, :], pattern=[[-30, 4]], base=0, channel_multiplier=1)
    nc.gpsimd.iota(si[:, :], pattern=[[1, CP_FULL]], base=0, channel_multiplier=-30)
    nc.vector.tensor_copy(out=mf[:, :], in_=mi[:, :])
    nc.vector.tensor_copy(out=sf[:, :], in_=si[:, :])
    for dst, src in ((mask, mf), (sel, sf)):
        ge = consts.tile(list(dst.shape), f32, tag=f"ge{dst.shape[0]}")
        lt = consts.tile(list(dst.shape), f32, tag=f"lt{dst.shape[0]}")
        nc.vector.tensor_scalar(out=ge[:, :], in0=src[:, :], scalar1=0.0,
                                scalar2=None, op0=Alu.is_ge)
        nc.vector.tensor_scalar(out=lt[:, :], in0=src[:, :], scalar1=30.0,
                                scalar2=None, op0=Alu.is_lt)
        nc.vector.tensor_tensor(out=dst[:, :], in0=ge[:, :], in1=lt[:, :],
                                op=Alu.mult)

    for s in range(NSAMP):
        # ---- im2col DMAs: imA [128, (d,h',w')], imB [16, (d,h',w')] ----
        imA = imc_pool.tile([128, IC, HC, WC], f32, tag="imA")
        imB = imc_pool.tile([16, IC, HC, WC], f32, tag="imB")
        xoff = x_ap.offset + s * (H * WD * IC * IA)
        for kh in range(K):
            for kw in range(K):
                blk = kh * K + kw
                for d in range(IC):
                    src = bass.AP(tensor=x_ap.tensor,
                                  offset=xoff + d * (IA * H * WD) + kh * WD + kw,
                                  ap=[[H * WD, IA], [WD, HC], [1, WC]])
                    if blk < 8:
                        nc.sync.dma_start(out=imA[blk * 16:(blk + 1) * 16, d, :, :], in_=src)
                    else:
                        nc.sync.dma_start(out=imB[:, d, :, :], in_=src)

        for (c, cp, nj) in CHUNKS:
            # ---- conv for this chunk: votes V [cp, (i,o,a)] ----
            V = v_pool.tile([CP_FULL, IC, OC, NA], f32, tag="V")
            for d in range(IC):
                pc = psum_c.tile([CP_FULL, CO], f32, tag="pc")
                nc.tensor.matmul(pc[:cp, :], imA[:, d, 4 * c:4 * c + nj, :],
                                 wa[:, :], start=True, stop=False)
                nc.tensor.matmul(pc[:cp, :], imB[:, d, 4 * c:4 * c + nj, :],
                                 wb[:, :], start=False, stop=True)
                nc.scalar.copy(out=V[:cp, d, :, :], in_=pc[:cp, :])

            # ---- routing ----
            L = small.tile([CP_FULL, IC, OC], f32, tag="L")
            nc.gpsimd.memset(L[:cp, :, :], 0.0)
            for it in range(ROUTINGS):
                # softmax over o (free)
                e = small.tile([CP_FULL, IC, OC], f32, tag="e")
                nc.scalar.activation(out=e[:cp, :, :], in_=L[:cp, :, :], func=Act.Exp,
                                     bias=zero_t[:cp, :])
                ssum = small.tile([CP_FULL, IC], f32, tag="ssum")
                nc.vector.tensor_reduce(out=ssum[:cp, :], in_=e[:cp, :, :],
                                        axis=X, op=Alu.add)
                srec = small.tile([CP_FULL, IC], f32, tag="srec")
                nc.vector.reciprocal(out=srec[:cp, :], in_=ssum[:cp, :])
                r = small.tile([CP_FULL, IC, OC], f32, tag="r")
                # iterate (o, i): innermost steps nonzero on all operands
                nc.vector.tensor_tensor(
                    out=pap(r, cp, [[1, OC], [OC, IC]]),
                    in0=pap(e, cp, [[1, OC], [OC, IC]]),
                    in1=pap(srec, cp, [[0, OC], [1, IC]]),
                    op=Alu.mult)
                # vr = V * r (broadcast over a); iterate (a, i, o)
                vr = vr_pool.tile([CP_FULL, IC, OC, NA], f32, tag="vr")
                nc.vector.tensor_tensor(
                    out=pap(vr, cp, [[1, NA], [CO, IC], [NA, OC]]),
                    in0=pap(V, cp, [[1, NA], [CO, IC], [NA, OC]]),
                    in1=pap(r, cp, [[0, NA], [OC, IC], [1, OC]]),
                    op=Alu.mult)
                # preact = sum_i vr  [cp, (o,a)]; reduce innermost=i
                preact = acts.tile([CP_FULL, CO], f32, tag="preact")
                nc.vector.tensor_reduce(
                    out=preact[:cp, :],
                    in_=pap(vr, cp, [[NA, OC], [1, NA], [CO, IC]]),
                    axis=X, op=Alu.add)
                nc.vector.tensor_tensor(out=preact[:cp, :], in0=preact[:cp, :],
                                        in1=bfull[:cp, :], op=Alu.add)
                # squash over w
                sq = acts.tile([CP_FULL, CO], f32, tag="sq")
                nc.scalar.activation(out=sq[:cp, :], in_=preact[:cp, :],
                                     func=Act.Square, bias=zero_t[:cp, :])
                s2 = psum_s.tile([4, CO], f32, tag="s2")
                nc.tensor.matmul(s2[:nj, :], mask[:cp, :nj], sq[:cp, :],
                                 start=True, stop=True)
                sqrt1 = small.tile([4, CO], f32, tag="sqrt1")
                nc.scalar.activation(out=sqrt1[:nj, :], in_=s2[:nj, :],
                                     func=Act.Sqrt, bias=eps_t[:nj, :])
                den = small.tile([4, CO], f32, tag="den")
                nc.vector.scalar_tensor_tensor(out=den[:nj, :], in0=s2[:nj, :],
                                               scalar=1.0, in1=sqrt1[:nj, :],
                                               op0=Alu.add, op1=Alu.mult)
                rden = small.tile([4, CO], f32, tag="rden")
                nc.vector.reciprocal(out=rden[:nj, :], in_=den[:nj, :])
                scl = small.tile([4, CO], f32, tag="scl")
                nc.vector.tensor_tensor(out=scl[:nj, :], in0=s2[:nj, :],
                                        in1=rden[:nj, :], op=Alu.mult)
                sclb = psum_b.tile([CP_FULL, CO], f32, tag="sclb")
                nc.tensor.matmul(sclb[:cp, :], sel[:nj, :cp], scl[:nj, :],
                                 start=True, stop=True)
                act = acts.tile([CP_FULL, CO], f32, tag="act")
                nc.vector.tensor_tensor(out=act[:cp, :], in0=preact[:cp, :],
                                        in1=sclb[:cp, :], op=Alu.mult)
                if it < ROUTINGS - 1:
                    # va = V * act (broadcast over i); natural (i,o,a) order
                    va = vr_pool.tile([CP_FULL, IC, OC, NA], f32, tag="vr")
                    nc.gpsimd.tensor_tensor(
                        out=va[:cp, :, :, :],
                        in0=V[:cp, :, :, :],
                        in1=pap(act, cp, [[0, IC], [NA, OC], [1, NA]]),
                        op=Alu.mult)
                    ld = small.tile([CP_FULL, IC, OC], f32, tag="ld")
                    nc.vector.tensor_reduce(out=ld[:cp, :, :], in_=va[:cp, :, :, :],
                                            axis=X, op=Alu.add)
                    nc.gpsimd.tensor_tensor(out=L[:cp, :, :], in0=L[:cp, :, :],
                                            in1=ld[:cp, :, :], op=Alu.add)
                else:
                    dst = bass.AP(tensor=out_ap.tensor,
                                  offset=out_ap.offset + s * (HW * CO) + c * (CP_FULL * CO),
                                  ap=[[CO, cp], [1, CO]])
                    nc.sync.dma_start(out=dst, in_=act[:cp, :])


_CACHED = None


def _build():
    global _CACHED
    if _CACHED is not None:
        return _CACHED
    from contextlib import ExitStack
    import concourse.bacc as bacc
    import concourse.mybir as mybir
    import concourse.tile as tile

    nc = bacc.Bacc("TRN2", target_bir_lowering=False, debug=False,
                   num_devices=NCORES)
    f32 = mybir.dt.float32
    x_t = nc.dram_tensor("x", [NSAMP, H, WD, IC, IA], f32, kind="ExternalInput")
    w_t = nc.dram_tensor("W", [IA, K, K, 1, CO], f32, kind="ExternalInput")
    b_t = nc.dram_tensor("b", [OC, NA, 1, 1], f32, kind="ExternalInput")
    out_t = nc.dram_tensor("out", [NSAMP, HC, WC, OC, NA], f32, kind="ExternalOutput")

    with tile.TileContext(nc) as tc:
        with ExitStack() as ctx:
            _build_body(ctx, tc, x_t.ap(), w_t.ap(), b_t.ap(), out_t.ap())
    nc.compile()
    _CACHED = nc
    return nc


def run(x, W, b, trace=False):
    from concourse.bass_utils import run_bass_kernel_spmd

    nc = _build()
    x = np.ascontiguousarray(x, np.float32)
    W = np.ascontiguousarray(W, np.float32)
    b = np.ascontiguousarray(b, np.float32)
    in_maps = [{"x": x[k * NSAMP:(k + 1) * NSAMP], "W": W, "b": b}
               for k in range(NCORES)]
    res = run_bass_kernel_spmd(nc, in_maps, core_ids=list(range(NCORES)),
                               trace=trace)
    out = np.concatenate([res.results[k]["out"] for k in range(NCORES)], axis=0)
    return out, res


def kernel(x, W, b):
    out, _ = run(x, W, b, trace=False)
    return out.astype(np.float32)


if __name__ == "__main__":
    nc = _build()
    print("built ok")

